# revision 15
# baseline (speedup 1.0000x reference)
"""Trainium2 Bass kernel for the MANE multi-view SGNS embedding loss.

Strategy: data-parallel over the batch axis B across 8 NeuronCores with the
embedding tables replicated per core (each core sees the two tables
concatenated into one [6*N, D] DRAM tensor).

The loss is pure embedding-gather + per-row dot products, so the kernel is
HBM-gather bound: ~352 MB of random 512 B rows per core.  The critical
resource is the SWDGE (gpsimd/Pool) descriptor-generation pipeline, whose
cost is ~994 ns fixed + 0.34 ns/row per indirect DMA call.  We therefore
batch gathers: one indirect_dma_start per (term, chunk) fetches
jb*(K+1) rows per partition (negatives + positive interleaved per batch
element), ~11k descriptors per call, so Pool-engine time stays ~0.3 ms while
the 16 SDMA engines stream at the ~360 GB/s HBM roofline (~1 ms).

Dot products: one 4D tensor_tensor (in-place) with the center row broadcast
over the K+1 gathered rows, then one tensor_reduce over D.  log-sigmoid sums
run on the scalar engine as Softplus with accum_out.  Per-core partials
[P, 2T] are combined on the host (scalar all-reduce).
"""

import ml_dtypes
import numpy as np

import concourse.bass as bass
import concourse.bacc as bacc
import concourse.tile as tile
from concourse import mybir
from concourse.bass_utils import run_bass_kernel_spmd

# ---------------------------------------------------------------- problem dims
V, N, D = 3, 200000, 128
B, K = 32768, 10
TOTAL = 65536
NCORES = 8
P = 128
T = 3 + 2 * V * (V - 1)  # 15 terms
NCHUNK = 4               # gather/compute chunks per core

F32 = mybir.dt.float32
BF16 = mybir.dt.bfloat16
I32 = mybir.dt.int32

# (j, i) pairs in reference order for cost2/cost3
PAIRS = [(j, i) for j in range(V) for i in range(V) if i != j]
# center view per term: cost1[i] -> i, cost2/3 (j,i) -> i
TERM_VIEW = [0, 1, 2] + [i for (_, i) in PAIRS] + [i for (_, i) in PAIRS]


def build_bass(bc, k, nchunk, n_rows=2 * V * N):
    """Build + compile the per-core Tile program.

    bc: batch elems per core; k: negatives per positive; nchunk: number of
    gather/compute chunks (each chunk covers bc//nchunk batch elems).
    """
    chunk = bc // nchunk
    jb = chunk // P           # batch elems per partition per chunk
    assert jb * P * nchunk == bc
    k1 = k + 1                # negs + positive per batch elem
    m = jb * k1               # gathered rows per partition per chunk

    nc = bacc.Bacc("TRN2", target_bir_lowering=False, debug=False,
                   enable_asserts=False)

    W = nc.dram_tensor("w_all", [n_rows, D], BF16, kind="ExternalInput")
    cidx = nc.dram_tensor("cidx", [V, P, nchunk * jb], I32, kind="ExternalInput")
    gidx = nc.dram_tensor("gidx", [T, P, nchunk * m], I32, kind="ExternalInput")
    acc_out = nc.dram_tensor("acc", [P, 2 * T], F32, kind="ExternalOutput")

    from contextlib import ExitStack
    with tile.TileContext(nc) as tc, ExitStack() as ctx:
        cen_pool = ctx.enter_context(tc.tile_pool(name="cen", bufs=1))
        idx_pool = ctx.enter_context(tc.tile_pool(name="idx", bufs=2))
        gat_pool = ctx.enter_context(tc.tile_pool(name="gat", bufs=3))
        h_pool = ctx.enter_context(tc.tile_pool(name="h", bufs=2))
        x_pool = ctx.enter_context(tc.tile_pool(name="x", bufs=2))
        scr_pool = ctx.enter_context(tc.tile_pool(name="scr", bufs=2))
        out_pool = ctx.enter_context(tc.tile_pool(name="out", bufs=1))

        # ---- centers: gather node embeddings for each view, chunk-local tiles
        CEN = []  # CEN[v][c]: [P, jb*D]
        for v in range(V):
            cit = idx_pool.tile([P, nchunk * jb], I32, tag="cidx")
            nc.sync.dma_start(cit[:], cidx.ap()[v])
            tiles_v = []
            for c in range(nchunk):
                ct = cen_pool.tile([P, jb * D], BF16, tag=f"cen_{v}_{c}")
                nc.gpsimd.indirect_dma_start(
                    out=ct[:], out_offset=None,
                    in_=W.ap(),
                    in_offset=bass.IndirectOffsetOnAxis(
                        ap=cit[:, c * jb:(c + 1) * jb], axis=0),
                )
                tiles_v.append(ct)
            CEN.append(tiles_v)

        # ACC columns: [0:T] = sum softplus(neg dots), [T:2T] = sum softplus(-pos)
        ACC = out_pool.tile([P, 2 * T], F32)

        for t in range(T):
            iv = TERM_VIEW[t]
            git = idx_pool.tile([P, nchunk * m], I32, tag="gidx")
            nc.sync.dma_start(git[:], gidx.ap()[t])

            XD = x_pool.tile([P, nchunk * m], BF16, tag="XD")  # dots
            for c in range(nchunk):
                GAT = gat_pool.tile([P, m * D], BF16, tag="gat")
                nc.gpsimd.indirect_dma_start(
                    out=GAT[:], out_offset=None,
                    in_=W.ap(),
                    in_offset=bass.IndirectOffsetOnAxis(
                        ap=git[:, c * m:(c + 1) * m], axis=0),
                )
                # dots: GAT[p, j, kk, :] *= CEN[p, j, :]; reduce over D.
                # The multiply always runs on DVE (flat operands hit the 2x
                # packed mode there).  TENSOR_REDUCE has no packed uop on
                # this HW (measured 1x), and strided tensor_tensor folds
                # also run 1x on DVE - so for half the chunks the 128->32
                # fold-adds run on the otherwise-idle GpSimd engine
                # (software loops, stride-agnostic), leaving DVE a 4x
                # smaller 1x reduce.
                nc.vector.tensor_tensor(
                    out=GAT[:].rearrange("p (j k d) -> p j k d", j=jb, k=k1),
                    in0=GAT[:].rearrange("p (j k d) -> p j k d", j=jb, k=k1),
                    in1=CEN[iv][c][:].rearrange("p (j d) -> p j d", j=jb)
                        .unsqueeze(2).to_broadcast([P, jb, k1, D]),
                    op=mybir.AluOpType.mult)
                g3 = GAT[:].rearrange("p (m d) -> p m d", m=m)
                if c < nchunk // 2:
                    H1 = h_pool.tile([P, m * (D // 2)], BF16, tag="h1")
                    h13 = H1[:].rearrange("p (m d) -> p m d", m=m)
                    nc.gpsimd.tensor_tensor(
                        out=h13, in0=g3[:, :, 0:D // 2], in1=g3[:, :, D // 2:D],
                        op=mybir.AluOpType.add)
                    H2 = h_pool.tile([P, m * (D // 4)], BF16, tag="h2")
                    h23 = H2[:].rearrange("p (m d) -> p m d", m=m)
                    nc.gpsimd.tensor_tensor(
                        out=h23, in0=h13[:, :, 0:D // 4],
                        in1=h13[:, :, D // 4:D // 2],
                        op=mybir.AluOpType.add)
                    red_in = h23
                else:
                    red_in = g3
                with nc.allow_low_precision("bf16 dot rounding ok, tol 2e-2"):
                    nc.vector.tensor_reduce(
                        out=XD[:, c * m:(c + 1) * m],
                        in_=red_in,
                        axis=mybir.AxisListType.X, op=mybir.AluOpType.add)

            # ACC[:, t]   = sum log_sigmoid(-x_neg) = sum Ln(Sigmoid(-x_neg))
            # ACC[:, T+t] = sum log_sigmoid(+x_pos)
            xd3 = XD[:].rearrange("p (g k) -> p g k", k=k1)
            sn = scr_pool.tile([P, nchunk * jb * k], F32, tag="sn")
            nc.scalar.activation(
                out=sn[:].rearrange("p (g k) -> p g k", k=k),
                in_=xd3[:, :, 0:k], scale=-1.0,
                func=mybir.ActivationFunctionType.Sigmoid)
            sn2 = scr_pool.tile([P, nchunk * jb * k], F32, tag="sn2")
            nc.scalar.activation(
                out=sn2[:], in_=sn[:],
                func=mybir.ActivationFunctionType.Ln,
                accum_out=ACC[:, t:t + 1])
            sp = scr_pool.tile([P, nchunk * jb], F32, tag="sp")
            nc.scalar.activation(
                out=sp[:].unsqueeze(2),
                in_=xd3[:, :, k:k1],
                func=mybir.ActivationFunctionType.Sigmoid)
            sp2 = scr_pool.tile([P, nchunk * jb], F32, tag="sp2")
            nc.scalar.activation(
                out=sp2[:], in_=sp[:],
                func=mybir.ActivationFunctionType.Ln,
                accum_out=ACC[:, T + t:T + t + 1])

        nc.sync.dma_start(acc_out.ap(), ACC[:])

    nc.compile()
    return nc


_NC_CACHE = {}


def _get_nc(bc, k, nchunk):
    key = (bc, k, nchunk)
    if key not in _NC_CACHE:
        _NC_CACHE[key] = build_bass(bc, k, nchunk)
    return _NC_CACHE[key]


def _lay2(x, nchunk, jb):
    # x: [..., bc] -> [..., P, nchunk*jb] with elem c*(P*jb)+p*jb+j -> col c*jb+j
    lead = x.shape[:-1]
    return (x.reshape(*lead, nchunk, P, jb)
             .swapaxes(-3, -2)
             .reshape(*lead, P, nchunk * jb))


def _lay3(x, nchunk, jb, k1):
    # x: [..., bc, k1] -> [..., P, nchunk*jb*k1]
    lead = x.shape[:-2]
    return (x.reshape(*lead, nchunk, P, jb, k1)
             .reshape(*lead, nchunk, P, jb * k1)
             .swapaxes(-3, -2)
             .reshape(*lead, P, nchunk * jb * k1))


def host_prep(count, shuffle_indices, nodes_idx, neigh_idx,
              neg_idx1, neg_idx2, neg_idx3, node_W, neigh_W,
              n_cores=NCORES, nchunk=NCHUNK, b=B):
    """Compute per-core input maps + the W table. Pure numpy."""
    c0 = int(count)
    sh = np.asarray(shuffle_indices)[:, c0:c0 + b].astype(np.int64)
    nodes_sel = np.take_along_axis(np.asarray(nodes_idx).astype(np.int64), sh, axis=1)
    neigh_sel = np.take_along_axis(np.asarray(neigh_idx).astype(np.int64), sh, axis=1)
    neg1 = np.asarray(neg_idx1).astype(np.int64)[:, :b]
    neg2 = np.asarray(neg_idx2).astype(np.int64)[:, :, :b]
    neg3 = np.asarray(neg_idx3).astype(np.int64)[:, :, :b]

    node_W = np.ascontiguousarray(np.asarray(node_W), dtype=np.float32)
    neigh_W = np.ascontiguousarray(np.asarray(neigh_W), dtype=np.float32)
    n = node_W.shape[1]
    d = node_W.shape[2]
    W_all = np.concatenate(
        [node_W.reshape(V * n, d), neigh_W.reshape(V * n, d)],
        axis=0).astype(ml_dtypes.bfloat16)

    # per-term (pos_idx, neg_idx, view) with global row offsets
    pos_list, neg_list = [], []
    for i in range(V):
        pos_list.append(neigh_sel[i] + (V + i) * n)
        neg_list.append(neg1[i] + (V + i) * n)
    for (j, i) in PAIRS:
        pos_list.append(nodes_sel[i] + j * n)
        neg_list.append(neg2[j, i] + j * n)
    for (j, i) in PAIRS:
        pos_list.append(neigh_sel[i] + (V + j) * n)
        neg_list.append(neg3[j, i] + (V + j) * n)
    pos_all = np.stack(pos_list)          # [T, b]
    neg_all = np.stack(neg_list)          # [T, b, K]
    cen_all = nodes_sel + (np.arange(V) * n)[:, None]  # [V, b]

    # combined per batch elem: K negs then the positive -> [T, b, K+1]
    comb = np.concatenate([neg_all, pos_all[:, :, None]], axis=2)

    bc = b // n_cores
    chunk = bc // nchunk
    jb = chunk // P
    k = neg_all.shape[-1]

    in_maps = []
    for core in range(n_cores):
        sl = slice(core * bc, (core + 1) * bc)
        in_maps.append({
            "w_all": W_all,
            "cidx": _lay2(cen_all[:, sl], nchunk, jb).astype(np.int32),
            "gidx": _lay3(comb[:, sl], nchunk, jb, k + 1).astype(np.int32),
        })
    return in_maps


def host_combine(acc_list, hyp1, hyp2, b=B):
    """acc_list: per-core [P, 2T] log-sigmoid-sum partials -> final scalar."""
    s = np.zeros(T, dtype=np.float64)
    for a in acc_list:
        a = np.asarray(a, dtype=np.float64).sum(axis=0)
        s += a[:T] + a[T:2 * T]
    term_val = s / b                       # sum of log-sigmoids per term
    cost1 = term_val[0:3].mean()
    cost2 = float(np.asarray(hyp1).reshape(-1)[0]) * term_val[3:9].sum() / 6.0
    cost3 = float(np.asarray(hyp2).reshape(-1)[0]) * term_val[9:15].sum() / 6.0
    return np.array(-(cost1 + cost2 + cost3) / 3.0, dtype=np.float32)


def kernel(count, shuffle_indices, nodes_idx, neigh_idx,
           neg_idx1, neg_idx2, neg_idx3, node_W, neigh_W, hyp1, hyp2):
    in_maps = host_prep(count, shuffle_indices, nodes_idx, neigh_idx,
                        neg_idx1, neg_idx2, neg_idx3, node_W, neigh_W)
    nc = _get_nc(B // NCORES, K, NCHUNK)
    res = run_bass_kernel_spmd(nc, in_maps, core_ids=list(range(NCORES)))
    acc_list = [r["acc"] for r in res.results]
    return host_combine(acc_list, hyp1, hyp2)


# revision 20
# speedup vs baseline: 1.1379x; 1.1379x over previous
"""Trainium2 Bass kernel for the MANE multi-view SGNS embedding loss.

Strategy: data-parallel over the batch axis B across 8 NeuronCores with the
embedding tables replicated per core (each core sees the two tables
concatenated into one [6*N, D] DRAM tensor).

The loss is pure embedding-gather + per-row dot products, so the kernel is
HBM-gather bound: ~352 MB of random 512 B rows per core.  The critical
resource is the SWDGE (gpsimd/Pool) descriptor-generation pipeline, whose
cost is ~994 ns fixed + 0.34 ns/row per indirect DMA call.  We therefore
batch gathers: one indirect_dma_start per (term, chunk) fetches
jb*(K+1) rows per partition (negatives + positive interleaved per batch
element), ~11k descriptors per call, so Pool-engine time stays ~0.3 ms while
the 16 SDMA engines stream at the ~360 GB/s HBM roofline (~1 ms).

Dot products: one 4D tensor_tensor (in-place) with the center row broadcast
over the K+1 gathered rows, then one tensor_reduce over D.  log-sigmoid sums
run on the scalar engine as Softplus with accum_out.  Per-core partials
[P, 2T] are combined on the host (scalar all-reduce).
"""

import ml_dtypes
import numpy as np

import concourse.bass as bass
import concourse.bacc as bacc
import concourse.tile as tile
from concourse import mybir
from concourse.bass_utils import run_bass_kernel_spmd

# ---------------------------------------------------------------- problem dims
V, N, D = 3, 200000, 128
B, K = 32768, 10
TOTAL = 65536
NCORES = 8
P = 128
T = 3 + 2 * V * (V - 1)  # 15 terms
NCHUNK = 4               # gather/compute chunks per core

F32 = mybir.dt.float32
BF16 = mybir.dt.bfloat16
I32 = mybir.dt.int32

# (j, i) pairs in reference order for cost2/cost3
PAIRS = [(j, i) for j in range(V) for i in range(V) if i != j]
# center view per term: cost1[i] -> i, cost2/3 (j,i) -> i
TERM_VIEW = [0, 1, 2] + [i for (_, i) in PAIRS] + [i for (_, i) in PAIRS]


def build_bass(bc, k, nchunk, n_rows=2 * V * N):
    """Build + compile the per-core Tile program.

    bc: batch elems per core; k: negatives per positive; nchunk: number of
    gather/compute chunks (each chunk covers bc//nchunk batch elems).
    """
    chunk = bc // nchunk
    jb = chunk // P           # batch elems per partition per chunk
    assert jb * P * nchunk == bc
    k1 = k + 1                # negs + positive per batch elem
    m = jb * k1               # gathered rows per partition per chunk

    nc = bacc.Bacc("TRN2", target_bir_lowering=False, debug=False,
                   enable_asserts=False)

    # The table is viewed as half-rows [2*n_rows, D/2]; the host doubles all
    # index values.  Each row is gathered as two calls (element_offset 0 and
    # D/2) into SEPARATE flat tiles, so the lo/hi product tiles can be
    # combined by a flat tensor_tensor ADD that runs in the DVE 2x packed
    # mode -- halving the 1x-only TENSOR_REDUCE reads.
    h = D // 2
    W = nc.dram_tensor("w_all", [2 * n_rows, h], BF16, kind="ExternalInput")
    cidx = nc.dram_tensor("cidx", [V, P, nchunk * jb], I32, kind="ExternalInput")
    gidx = nc.dram_tensor("gidx", [T, P, nchunk * m], I32, kind="ExternalInput")
    acc_out = nc.dram_tensor("acc", [P, 2 * T], F32, kind="ExternalOutput")

    from contextlib import ExitStack
    with tile.TileContext(nc) as tc, ExitStack() as ctx:
        cen_pool = ctx.enter_context(tc.tile_pool(name="cen", bufs=1))
        idx_pool = ctx.enter_context(tc.tile_pool(name="idx", bufs=2))
        gat_pool = ctx.enter_context(tc.tile_pool(name="gat", bufs=3))
        h_pool = ctx.enter_context(tc.tile_pool(name="h", bufs=2))
        x_pool = ctx.enter_context(tc.tile_pool(name="x", bufs=2))
        scr_pool = ctx.enter_context(tc.tile_pool(name="scr", bufs=2))
        out_pool = ctx.enter_context(tc.tile_pool(name="out", bufs=1))

        # ---- centers: gather node embeddings for each view, chunk-local
        # lo/hi half-row tiles
        CEN = []  # CEN[v][c]: ([P, jb*h] lo, [P, jb*h] hi)
        for v in range(V):
            cit = idx_pool.tile([P, nchunk * jb], I32, tag="cidx")
            nc.sync.dma_start(cit[:], cidx.ap()[v])
            tiles_v = []
            for c in range(nchunk):
                pair = []
                for half in range(2):
                    ct = cen_pool.tile([P, jb * h], BF16,
                                       tag=f"cen_{v}_{c}_{half}")
                    nc.gpsimd.indirect_dma_start(
                        out=ct[:], out_offset=None,
                        in_=W.ap(),
                        in_offset=bass.IndirectOffsetOnAxis(
                            ap=cit[:, c * jb:(c + 1) * jb], axis=0),
                        element_offset=half * h,
                    )
                    pair.append(ct)
                tiles_v.append(pair)
            CEN.append(tiles_v)

        # ACC columns: [0:T] = sum softplus(neg dots), [T:2T] = sum softplus(-pos)
        ACC = out_pool.tile([P, 2 * T], F32)

        for t in range(T):
            iv = TERM_VIEW[t]
            git = idx_pool.tile([P, nchunk * m], I32, tag="gidx")
            nc.sync.dma_start(git[:], gidx.ap()[t])

            XD = x_pool.tile([P, nchunk * m], BF16, tag="XD")  # dots
            for c in range(nchunk):
                GLO = gat_pool.tile([P, m * h], BF16, tag="glo")
                GHI = gat_pool.tile([P, m * h], BF16, tag="ghi")
                for half, gt in ((0, GLO), (1, GHI)):
                    nc.gpsimd.indirect_dma_start(
                        out=gt[:], out_offset=None,
                        in_=W.ap(),
                        in_offset=bass.IndirectOffsetOnAxis(
                            ap=git[:, c * m:(c + 1) * m], axis=0),
                        element_offset=half * h,
                    )
                # half-products (flat in0/out + broadcast in1 -> DVE 2x).
                # A few chunks multiply on the otherwise-idle GpSimd to
                # offload the critical DVE.
                eng = (nc.gpsimd if (c == nchunk - 1 and t % 2 == 1)
                       else nc.vector)
                for gt, ct in ((GLO, CEN[iv][c][0]), (GHI, CEN[iv][c][1])):
                    eng.tensor_tensor(
                        out=gt[:].rearrange("p (j k d) -> p j k d", j=jb, k=k1),
                        in0=gt[:].rearrange("p (j k d) -> p j k d", j=jb, k=k1),
                        in1=ct[:].rearrange("p (j d) -> p j d", j=jb)
                            .unsqueeze(2).to_broadcast([P, jb, k1, h]),
                        op=mybir.AluOpType.mult)
                # flat fold (all-flat operands -> DVE 2x), then a half-size
                # 1x reduce (TENSOR_REDUCE has no packed uop on this HW).
                nc.vector.tensor_tensor(
                    out=GLO[:], in0=GLO[:], in1=GHI[:],
                    op=mybir.AluOpType.add)
                with nc.allow_low_precision("bf16 dot rounding ok, tol 2e-2"):
                    nc.vector.tensor_reduce(
                        out=XD[:, c * m:(c + 1) * m],
                        in_=GLO[:].rearrange("p (m d) -> p m d", m=m),
                        axis=mybir.AxisListType.X, op=mybir.AluOpType.add)

            # ACC[:, t]   = sum log_sigmoid(-x_neg) = sum Ln(Sigmoid(-x_neg))
            # ACC[:, T+t] = sum log_sigmoid(+x_pos)
            xd3 = XD[:].rearrange("p (g k) -> p g k", k=k1)
            sn = scr_pool.tile([P, nchunk * jb * k], F32, tag="sn")
            nc.scalar.activation(
                out=sn[:].rearrange("p (g k) -> p g k", k=k),
                in_=xd3[:, :, 0:k], scale=-1.0,
                func=mybir.ActivationFunctionType.Sigmoid)
            sn2 = scr_pool.tile([P, nchunk * jb * k], F32, tag="sn2")
            nc.scalar.activation(
                out=sn2[:], in_=sn[:],
                func=mybir.ActivationFunctionType.Ln,
                accum_out=ACC[:, t:t + 1])
            sp = scr_pool.tile([P, nchunk * jb], F32, tag="sp")
            nc.scalar.activation(
                out=sp[:].unsqueeze(2),
                in_=xd3[:, :, k:k1],
                func=mybir.ActivationFunctionType.Sigmoid)
            sp2 = scr_pool.tile([P, nchunk * jb], F32, tag="sp2")
            nc.scalar.activation(
                out=sp2[:], in_=sp[:],
                func=mybir.ActivationFunctionType.Ln,
                accum_out=ACC[:, T + t:T + t + 1])

        nc.sync.dma_start(acc_out.ap(), ACC[:])

    nc.compile()
    return nc


_NC_CACHE = {}


def _get_nc(bc, k, nchunk):
    key = (bc, k, nchunk)
    if key not in _NC_CACHE:
        _NC_CACHE[key] = build_bass(bc, k, nchunk)
    return _NC_CACHE[key]


def _lay2(x, nchunk, jb):
    # x: [..., bc] -> [..., P, nchunk*jb] with elem c*(P*jb)+p*jb+j -> col c*jb+j
    lead = x.shape[:-1]
    return (x.reshape(*lead, nchunk, P, jb)
             .swapaxes(-3, -2)
             .reshape(*lead, P, nchunk * jb))


def _lay3(x, nchunk, jb, k1):
    # x: [..., bc, k1] -> [..., P, nchunk*jb*k1]
    lead = x.shape[:-2]
    return (x.reshape(*lead, nchunk, P, jb, k1)
             .reshape(*lead, nchunk, P, jb * k1)
             .swapaxes(-3, -2)
             .reshape(*lead, P, nchunk * jb * k1))


def host_prep(count, shuffle_indices, nodes_idx, neigh_idx,
              neg_idx1, neg_idx2, neg_idx3, node_W, neigh_W,
              n_cores=NCORES, nchunk=NCHUNK, b=B):
    """Compute per-core input maps + the W table. Pure numpy."""
    c0 = int(count)
    sh = np.asarray(shuffle_indices)[:, c0:c0 + b].astype(np.int64)
    nodes_sel = np.take_along_axis(np.asarray(nodes_idx).astype(np.int64), sh, axis=1)
    neigh_sel = np.take_along_axis(np.asarray(neigh_idx).astype(np.int64), sh, axis=1)
    neg1 = np.asarray(neg_idx1).astype(np.int64)[:, :b]
    neg2 = np.asarray(neg_idx2).astype(np.int64)[:, :, :b]
    neg3 = np.asarray(neg_idx3).astype(np.int64)[:, :, :b]

    node_W = np.ascontiguousarray(np.asarray(node_W), dtype=np.float32)
    neigh_W = np.ascontiguousarray(np.asarray(neigh_W), dtype=np.float32)
    n = node_W.shape[1]
    d = node_W.shape[2]
    # table viewed as half-rows [2*rows, d/2]; all indices are doubled
    W_all = np.concatenate(
        [node_W.reshape(V * n, d), neigh_W.reshape(V * n, d)],
        axis=0).astype(ml_dtypes.bfloat16).reshape(2 * V * n * 2, d // 2)

    # per-term (pos_idx, neg_idx, view) with global row offsets
    pos_list, neg_list = [], []
    for i in range(V):
        pos_list.append(neigh_sel[i] + (V + i) * n)
        neg_list.append(neg1[i] + (V + i) * n)
    for (j, i) in PAIRS:
        pos_list.append(nodes_sel[i] + j * n)
        neg_list.append(neg2[j, i] + j * n)
    for (j, i) in PAIRS:
        pos_list.append(neigh_sel[i] + (V + j) * n)
        neg_list.append(neg3[j, i] + (V + j) * n)
    pos_all = np.stack(pos_list)          # [T, b]
    neg_all = np.stack(neg_list)          # [T, b, K]
    cen_all = nodes_sel + (np.arange(V) * n)[:, None]  # [V, b]

    # combined per batch elem: K negs then the positive -> [T, b, K+1]
    comb = np.concatenate([neg_all, pos_all[:, :, None]], axis=2)

    bc = b // n_cores
    chunk = bc // nchunk
    jb = chunk // P
    k = neg_all.shape[-1]

    in_maps = []
    for core in range(n_cores):
        sl = slice(core * bc, (core + 1) * bc)
        in_maps.append({
            "w_all": W_all,
            "cidx": (2 * _lay2(cen_all[:, sl], nchunk, jb)).astype(np.int32),
            "gidx": (2 * _lay3(comb[:, sl], nchunk, jb, k + 1)).astype(np.int32),
        })
    return in_maps


def host_combine(acc_list, hyp1, hyp2, b=B):
    """acc_list: per-core [P, 2T] log-sigmoid-sum partials -> final scalar."""
    s = np.zeros(T, dtype=np.float64)
    for a in acc_list:
        a = np.asarray(a, dtype=np.float64).sum(axis=0)
        s += a[:T] + a[T:2 * T]
    term_val = s / b                       # sum of log-sigmoids per term
    cost1 = term_val[0:3].mean()
    cost2 = float(np.asarray(hyp1).reshape(-1)[0]) * term_val[3:9].sum() / 6.0
    cost3 = float(np.asarray(hyp2).reshape(-1)[0]) * term_val[9:15].sum() / 6.0
    return np.array(-(cost1 + cost2 + cost3) / 3.0, dtype=np.float32)


def kernel(count, shuffle_indices, nodes_idx, neigh_idx,
           neg_idx1, neg_idx2, neg_idx3, node_W, neigh_W, hyp1, hyp2):
    in_maps = host_prep(count, shuffle_indices, nodes_idx, neigh_idx,
                        neg_idx1, neg_idx2, neg_idx3, node_W, neigh_W)
    nc = _get_nc(B // NCORES, K, NCHUNK)
    res = run_bass_kernel_spmd(nc, in_maps, core_ids=list(range(NCORES)))
    acc_list = [r["acc"] for r in res.results]
    return host_combine(acc_list, hyp1, hyp2)


# revision 22
# speedup vs baseline: 1.1811x; 1.0380x over previous
"""Trainium2 Bass kernel for the MANE multi-view SGNS embedding loss.

Strategy: data-parallel over the batch axis B across 8 NeuronCores with the
embedding tables replicated per core (each core sees the two tables
concatenated into one [6*N, D] DRAM tensor).

The loss is pure embedding-gather + per-row dot products, so the kernel is
HBM-gather bound: ~352 MB of random 512 B rows per core.  The critical
resource is the SWDGE (gpsimd/Pool) descriptor-generation pipeline, whose
cost is ~994 ns fixed + 0.34 ns/row per indirect DMA call.  We therefore
batch gathers: one indirect_dma_start per (term, chunk) fetches
jb*(K+1) rows per partition (negatives + positive interleaved per batch
element), ~11k descriptors per call, so Pool-engine time stays ~0.3 ms while
the 16 SDMA engines stream at the ~360 GB/s HBM roofline (~1 ms).

Dot products: one 4D tensor_tensor (in-place) with the center row broadcast
over the K+1 gathered rows, then one tensor_reduce over D.  log-sigmoid sums
run on the scalar engine as Softplus with accum_out.  Per-core partials
[P, 2T] are combined on the host (scalar all-reduce).
"""

import ml_dtypes
import numpy as np

import concourse.bass as bass
import concourse.bacc as bacc
import concourse.tile as tile
from concourse import mybir
from concourse.bass_utils import run_bass_kernel_spmd

# ---------------------------------------------------------------- problem dims
V, N, D = 3, 200000, 128
B, K = 32768, 10
TOTAL = 65536
NCORES = 8
P = 128
T = 3 + 2 * V * (V - 1)  # 15 terms
NCHUNK = 4               # gather/compute chunks per core

F32 = mybir.dt.float32
BF16 = mybir.dt.bfloat16
I32 = mybir.dt.int32

# (j, i) pairs in reference order for cost2/cost3
PAIRS = [(j, i) for j in range(V) for i in range(V) if i != j]
# center view per term: cost1[i] -> i, cost2/3 (j,i) -> i
TERM_VIEW = [0, 1, 2] + [i for (_, i) in PAIRS] + [i for (_, i) in PAIRS]


def build_bass(bc, k, nchunk, n_rows=2 * V * N):
    """Build + compile the per-core Tile program.

    bc: batch elems per core; k: negatives per positive; nchunk: number of
    gather/compute chunks (each chunk covers bc//nchunk batch elems).
    """
    chunk = bc // nchunk
    jb = chunk // P           # batch elems per partition per chunk
    assert jb * P * nchunk == bc
    k1 = k + 1                # negs + positive per batch elem
    m = jb * k1               # gathered rows per partition per chunk

    nc = bacc.Bacc("TRN2", target_bir_lowering=False, debug=False,
                   enable_asserts=False)

    # The table is viewed as half-rows [2*n_rows, D/2]; the host doubles all
    # index values.  Each row is gathered as two calls (element_offset 0 and
    # D/2) into SEPARATE flat tiles, so the lo/hi product tiles can be
    # combined by a flat tensor_tensor ADD that runs in the DVE 2x packed
    # mode -- halving the 1x-only TENSOR_REDUCE reads.
    h = D // 2
    W = nc.dram_tensor("w_all", [2 * n_rows, h], BF16, kind="ExternalInput")
    cidx = nc.dram_tensor("cidx", [V, P, nchunk * jb], I32, kind="ExternalInput")
    gidx = nc.dram_tensor("gidx", [T, P, nchunk * m], I32, kind="ExternalInput")
    acc_out = nc.dram_tensor("acc", [P, 2 * T], F32, kind="ExternalOutput")

    from contextlib import ExitStack
    with tile.TileContext(nc) as tc, ExitStack() as ctx:
        cen_pool = ctx.enter_context(tc.tile_pool(name="cen", bufs=1))
        idx_pool = ctx.enter_context(tc.tile_pool(name="idx", bufs=2))
        gat_pool = ctx.enter_context(tc.tile_pool(name="gat", bufs=4))
        h_pool = ctx.enter_context(tc.tile_pool(name="h", bufs=2))
        x_pool = ctx.enter_context(tc.tile_pool(name="x", bufs=2))
        scr_pool = ctx.enter_context(tc.tile_pool(name="scr", bufs=2))
        out_pool = ctx.enter_context(tc.tile_pool(name="out", bufs=1))

        # ---- centers: gather node embeddings for each view, chunk-local
        # lo/hi half-row tiles
        CEN = []  # CEN[v][c]: ([P, jb*h] lo, [P, jb*h] hi)
        for v in range(V):
            cit = idx_pool.tile([P, nchunk * jb], I32, tag="cidx")
            nc.sync.dma_start(cit[:], cidx.ap()[v])
            tiles_v = []
            for c in range(nchunk):
                pair = []
                for half in range(2):
                    ct = cen_pool.tile([P, jb * h], BF16,
                                       tag=f"cen_{v}_{c}_{half}")
                    nc.gpsimd.indirect_dma_start(
                        out=ct[:], out_offset=None,
                        in_=W.ap(),
                        in_offset=bass.IndirectOffsetOnAxis(
                            ap=cit[:, c * jb:(c + 1) * jb], axis=0),
                        element_offset=half * h,
                    )
                    pair.append(ct)
                tiles_v.append(pair)
            CEN.append(tiles_v)

        # ACC columns: [0:T] = sum softplus(neg dots), [T:2T] = sum softplus(-pos)
        ACC = out_pool.tile([P, 2 * T], F32)

        for t in range(T):
            iv = TERM_VIEW[t]
            git = idx_pool.tile([P, nchunk * m], I32, tag="gidx")
            nc.sync.dma_start(git[:], gidx.ap()[t])

            XD = x_pool.tile([P, nchunk * m], BF16, tag="XD")  # dots
            for c in range(nchunk):
                GLO = gat_pool.tile([P, m * h], BF16, tag="glo")
                GHI = gat_pool.tile([P, m * h], BF16, tag="ghi")
                for half, gt in ((0, GLO), (1, GHI)):
                    nc.gpsimd.indirect_dma_start(
                        out=gt[:], out_offset=None,
                        in_=W.ap(),
                        in_offset=bass.IndirectOffsetOnAxis(
                            ap=git[:, c * m:(c + 1) * m], axis=0),
                        element_offset=half * h,
                    )
                # half-products (flat in0/out + broadcast in1 -> DVE 2x).
                # A few chunks multiply on the otherwise-idle GpSimd to
                # offload the critical DVE.
                eng = (nc.gpsimd if (c == nchunk - 1 and t % 3 != 0)
                       else nc.vector)
                for gt, ct in ((GLO, CEN[iv][c][0]), (GHI, CEN[iv][c][1])):
                    eng.tensor_tensor(
                        out=gt[:].rearrange("p (j k d) -> p j k d", j=jb, k=k1),
                        in0=gt[:].rearrange("p (j k d) -> p j k d", j=jb, k=k1),
                        in1=ct[:].rearrange("p (j d) -> p j d", j=jb)
                            .unsqueeze(2).to_broadcast([P, jb, k1, h]),
                        op=mybir.AluOpType.mult)
                # flat fold (all-flat operands -> DVE 2x), then a half-size
                # 1x reduce (TENSOR_REDUCE has no packed uop on this HW).
                nc.vector.tensor_tensor(
                    out=GLO[:], in0=GLO[:], in1=GHI[:],
                    op=mybir.AluOpType.add)
                with nc.allow_low_precision("bf16 dot rounding ok, tol 2e-2"):
                    nc.vector.tensor_reduce(
                        out=XD[:, c * m:(c + 1) * m],
                        in_=GLO[:].rearrange("p (m d) -> p m d", m=m),
                        axis=mybir.AxisListType.X, op=mybir.AluOpType.add)

            # ACC[:, t]   = sum log_sigmoid(-x_neg) = sum Ln(Sigmoid(-x_neg))
            # ACC[:, T+t] = sum log_sigmoid(+x_pos)
            xd3 = XD[:].rearrange("p (g k) -> p g k", k=k1)
            sn = scr_pool.tile([P, nchunk * jb * k], F32, tag="sn")
            nc.scalar.activation(
                out=sn[:].rearrange("p (g k) -> p g k", k=k),
                in_=xd3[:, :, 0:k], scale=-1.0,
                func=mybir.ActivationFunctionType.Sigmoid)
            sn2 = scr_pool.tile([P, nchunk * jb * k], F32, tag="sn2")
            nc.scalar.activation(
                out=sn2[:], in_=sn[:],
                func=mybir.ActivationFunctionType.Ln,
                accum_out=ACC[:, t:t + 1])
            sp = scr_pool.tile([P, nchunk * jb], F32, tag="sp")
            nc.scalar.activation(
                out=sp[:].unsqueeze(2),
                in_=xd3[:, :, k:k1],
                func=mybir.ActivationFunctionType.Sigmoid)
            sp2 = scr_pool.tile([P, nchunk * jb], F32, tag="sp2")
            nc.scalar.activation(
                out=sp2[:], in_=sp[:],
                func=mybir.ActivationFunctionType.Ln,
                accum_out=ACC[:, T + t:T + t + 1])

        nc.sync.dma_start(acc_out.ap(), ACC[:])

    nc.compile()
    return nc


_NC_CACHE = {}


def _get_nc(bc, k, nchunk):
    key = (bc, k, nchunk)
    if key not in _NC_CACHE:
        _NC_CACHE[key] = build_bass(bc, k, nchunk)
    return _NC_CACHE[key]


def _lay2(x, nchunk, jb):
    # x: [..., bc] -> [..., P, nchunk*jb] with elem c*(P*jb)+p*jb+j -> col c*jb+j
    lead = x.shape[:-1]
    return (x.reshape(*lead, nchunk, P, jb)
             .swapaxes(-3, -2)
             .reshape(*lead, P, nchunk * jb))


def _lay3(x, nchunk, jb, k1):
    # x: [..., bc, k1] -> [..., P, nchunk*jb*k1]
    lead = x.shape[:-2]
    return (x.reshape(*lead, nchunk, P, jb, k1)
             .reshape(*lead, nchunk, P, jb * k1)
             .swapaxes(-3, -2)
             .reshape(*lead, P, nchunk * jb * k1))


def host_prep(count, shuffle_indices, nodes_idx, neigh_idx,
              neg_idx1, neg_idx2, neg_idx3, node_W, neigh_W,
              n_cores=NCORES, nchunk=NCHUNK, b=B):
    """Compute per-core input maps + the W table. Pure numpy."""
    c0 = int(count)
    sh = np.asarray(shuffle_indices)[:, c0:c0 + b].astype(np.int64)
    nodes_sel = np.take_along_axis(np.asarray(nodes_idx).astype(np.int64), sh, axis=1)
    neigh_sel = np.take_along_axis(np.asarray(neigh_idx).astype(np.int64), sh, axis=1)
    neg1 = np.asarray(neg_idx1).astype(np.int64)[:, :b]
    neg2 = np.asarray(neg_idx2).astype(np.int64)[:, :, :b]
    neg3 = np.asarray(neg_idx3).astype(np.int64)[:, :, :b]

    node_W = np.ascontiguousarray(np.asarray(node_W), dtype=np.float32)
    neigh_W = np.ascontiguousarray(np.asarray(neigh_W), dtype=np.float32)
    n = node_W.shape[1]
    d = node_W.shape[2]
    # table viewed as half-rows [2*rows, d/2]; all indices are doubled
    W_all = np.concatenate(
        [node_W.reshape(V * n, d), neigh_W.reshape(V * n, d)],
        axis=0).astype(ml_dtypes.bfloat16).reshape(2 * V * n * 2, d // 2)

    # per-term (pos_idx, neg_idx, view) with global row offsets
    pos_list, neg_list = [], []
    for i in range(V):
        pos_list.append(neigh_sel[i] + (V + i) * n)
        neg_list.append(neg1[i] + (V + i) * n)
    for (j, i) in PAIRS:
        pos_list.append(nodes_sel[i] + j * n)
        neg_list.append(neg2[j, i] + j * n)
    for (j, i) in PAIRS:
        pos_list.append(neigh_sel[i] + (V + j) * n)
        neg_list.append(neg3[j, i] + (V + j) * n)
    pos_all = np.stack(pos_list)          # [T, b]
    neg_all = np.stack(neg_list)          # [T, b, K]
    cen_all = nodes_sel + (np.arange(V) * n)[:, None]  # [V, b]

    # combined per batch elem: K negs then the positive -> [T, b, K+1]
    comb = np.concatenate([neg_all, pos_all[:, :, None]], axis=2)

    bc = b // n_cores
    chunk = bc // nchunk
    jb = chunk // P
    k = neg_all.shape[-1]

    in_maps = []
    for core in range(n_cores):
        sl = slice(core * bc, (core + 1) * bc)
        in_maps.append({
            "w_all": W_all,
            "cidx": (2 * _lay2(cen_all[:, sl], nchunk, jb)).astype(np.int32),
            "gidx": (2 * _lay3(comb[:, sl], nchunk, jb, k + 1)).astype(np.int32),
        })
    return in_maps


def host_combine(acc_list, hyp1, hyp2, b=B):
    """acc_list: per-core [P, 2T] log-sigmoid-sum partials -> final scalar."""
    s = np.zeros(T, dtype=np.float64)
    for a in acc_list:
        a = np.asarray(a, dtype=np.float64).sum(axis=0)
        s += a[:T] + a[T:2 * T]
    term_val = s / b                       # sum of log-sigmoids per term
    cost1 = term_val[0:3].mean()
    cost2 = float(np.asarray(hyp1).reshape(-1)[0]) * term_val[3:9].sum() / 6.0
    cost3 = float(np.asarray(hyp2).reshape(-1)[0]) * term_val[9:15].sum() / 6.0
    return np.array(-(cost1 + cost2 + cost3) / 3.0, dtype=np.float32)


def kernel(count, shuffle_indices, nodes_idx, neigh_idx,
           neg_idx1, neg_idx2, neg_idx3, node_W, neigh_W, hyp1, hyp2):
    in_maps = host_prep(count, shuffle_indices, nodes_idx, neigh_idx,
                        neg_idx1, neg_idx2, neg_idx3, node_W, neigh_W)
    nc = _get_nc(B // NCORES, K, NCHUNK)
    res = run_bass_kernel_spmd(nc, in_maps, core_ids=list(range(NCORES)))
    acc_list = [r["acc"] for r in res.results]
    return host_combine(acc_list, hyp1, hyp2)
